# revision 21
# baseline (speedup 1.0000x reference)
"""Multi-head attention (B=2, S=2048, H=1024, 16 heads) on 8 TRN2 NeuronCores.

Sharding (tensor-parallel heads x data-parallel batch, per the hint):
  core c -> batch b = c // 4, head group g = c % 4 (4 heads each).
Each core computes, for its batch and its 4 heads:
  Q^T, K^T (transposed layout, qcol on partitions, fp16, duplicated across
  both partition halves), V in natural [tok, vdim] layout (projected
  directly with xT token-chunks as the stationary operand -- no PE
  transpose pass), scores^T = K^T.T @ Q^T per head with two tok_k chunks
  row-tiled concurrently in the two array halves, probs = exp(scores)
  (no max subtraction -- scores ~ N(0,1), bounded), ctx^T via a
  128-column ones-augmented V stationary (cols 65..127 zero) so the
  softmax denominator lands in row 64 of the same accumulation, division
  by the denominator (reciprocal + gpsimd partition broadcast), and the
  partial out-projection ctx^T.T @ Wo_rows.  The 4 partial outputs per
  batch are summed on the host during unsharding (Megatron-style TP
  partial sums).

Phase plan (PE-dense from ~3us to keep the HAM clock warm):
  exp-table preload dummy, then j-major xT DMA so Q-proj starts on the
  first 1 MB; Q/K/V projections back to back; attention pipeline
  (scores+exp LEADs ctx by 2 chunk-pair steps); per-head division;
  pipelined out-projection with the output DMA spread over 3 queues.

Biases: bq/bk applied on-device, bv/bo folded into a host-side additive
constant (bv @ Wo + bo), which is exact.
"""

import ml_dtypes
import numpy as np

import concourse.bacc as bacc
import concourse.mybir as mybir
import concourse.tile as tile
from concourse.bass_utils import run_bass_kernel_spmd

NCORES = 8
B, S, HID = 2, 2048, 1024
NH, HD = 16, 64
HPC = 4            # heads per core
QC = HPC * HD      # 256 local projection cols per core
HC = HID // 128    # 8 hidden chunks
TC = S // 128      # 16 token chunks
TB = S // 512      # 4 token blocks

F32 = mybir.dt.float32
BF16 = mybir.dt.bfloat16
FP16 = mybir.dt.float16
EXP = mybir.ActivationFunctionType.Exp
MULT = mybir.AluOpType.mult


def build_nc():
    nc = bacc.Bacc("TRN2", target_bir_lowering=False, debug=False,
                   num_devices=NCORES)
    xT = nc.declare_dram_parameter("xT", [HID, S], FP16, isOutput=False)
    wq = nc.declare_dram_parameter("wq", [HID, QC], FP16, isOutput=False)
    wk = nc.declare_dram_parameter("wk", [HID, QC], FP16, isOutput=False)
    wv = nc.declare_dram_parameter("wv", [HID, QC], FP16, isOutput=False)
    wo = nc.declare_dram_parameter("wo", [QC, HID], BF16, isOutput=False)
    bq = nc.declare_dram_parameter("bq", [QC], F32, isOutput=False)
    bk = nc.declare_dram_parameter("bk", [QC], F32, isOutput=False)
    out = nc.declare_dram_parameter("out", [S, HID], BF16, isOutput=True)

    with tile.TileContext(nc) as tc:
        with (
            tc.tile_pool(name="const", bufs=1) as constp,
            tc.tile_pool(name="qkv", bufs=1) as qkvp,
        ):
            wo_sb = constp.tile([128, 2 * HID], BF16)
            bq_sb = constp.tile([128, 2], F32)
            bk_sb = constp.tile([128, 2], F32)
            dum_f = constp.tile([1, 16], F32)
            dum_o = constp.tile([1, 16], BF16)
            # preload the exp table set (~2.7us) under the DMA lead-in
            nc.vector.memset(dum_f[:], 0.0)
            nc.scalar.activation(dum_o[:], dum_f[:], EXP)
            # Q^T/K^T per head, duplicated across both partition halves:
            # head h occupies free range [h*S, (h+1)*S) with the same [64, S]
            # data in partitions 0-63 and 64-127, so the scores matmuls can
            # run two tok_k chunks concurrently as row-tiles.
            qt2 = qkvp.tile([128, HPC * S], FP16)
            kt2 = qkvp.tile([128, HPC * S], FP16)
            # Natural V (bf16): per (tok_chunk, head) a 128-wide strip:
            # cols 0..63 = V dims, col 64 = ones (softmax denominator row),
            # cols 65..127 = zeros.  Full-width stationary keeps FWL on.
            v_sb = qkvp.tile([128, TC * HPC * 128], BF16)
            ctxf_sb = qkvp.tile([128, 2 * S], BF16)

            nc.vector.memset(v_sb[:], 0.0)
            for t in range(TC):
                for h in range(HPC):
                    off = (t * HPC + h) * 128 + HD
                    nc.vector.memset(v_sb[:, off:off + 1], 1.0)

            # ---- phase 1: projections -------------------------------------
            with (
                tc.tile_pool(name="xw", bufs=1) as xwp,
                tc.tile_pool(name="ps1", bufs=2, space="PSUM") as ps1,
            ):
                xT_sb = xwp.tile([128, HC * S], FP16)
                wq_sb = xwp.tile([128, HC * QC], FP16)
                wk_sb = xwp.tile([128, HC * QC], FP16)
                wv_sb = xwp.tile([128, HC * QC], FP16)

                # weights for Q first (small), then xT j-major so the
                # j-block-0 slices of all 8 hidden chunks (1 MB) land first
                # and the first Q matmuls can start ~3us in.
                qk_dmas = []
                for hc in range(HC):
                    r = slice(hc * 128, (hc + 1) * 128)
                    qk_dmas.append(nc.sync.dma_start(
                        wq_sb[:, hc * QC:(hc + 1) * QC], wq[r, :]))
                    qk_dmas.append(nc.scalar.dma_start(
                        wk_sb[:, hc * QC:(hc + 1) * QC], wk[r, :]))
                xt_dmas = {}
                for j in range(TB):
                    for hc in range(HC):
                        r = slice(hc * 128, (hc + 1) * 128)
                        eng = nc.sync if hc % 2 == 0 else nc.scalar
                        xt_dmas[(j, hc)] = eng.dma_start(
                            xT_sb[:, hc * S + j * 512:hc * S + (j + 1) * 512],
                            xT[r, j * 512:(j + 1) * 512])
                for ci in range(2):
                    nc.sync.dma_start(bq_sb[:, ci:ci + 1],
                                      bq[ci * 128:(ci + 1) * 128])
                    nc.sync.dma_start(bk_sb[:, ci:ci + 1],
                                      bk[ci * 128:(ci + 1) * 128])
                wv_dmas = []
                for hc in range(HC):
                    r = slice(hc * 128, (hc + 1) * 128)
                    wv_dmas.append(nc.scalar.dma_start(
                        wv_sb[:, hc * QC:(hc + 1) * QC], wv[r, :]))

                # Q then K projections (transposed layout, per ci group),
                # written into the duplicated per-head qt2/kt2 layout.
                qk_mms = {}
                for w_sb, b_sb, dst, wname in ((wq_sb, bq_sb, qt2, "q"),
                                               (wk_sb, bk_sb, kt2, "k")):
                    for ci in range(2):
                        ps = ps1.tile([128, S], F32, tag="ps1")
                        # j-outer: each j-block's 8-chunk accumulation lives
                        # in its own psum bank, and the paced xT j-block
                        # DMAs get a full j-pass (~4.5us) of headroom.
                        for j in range(TB):
                            for hc in range(HC):
                                mm = nc.tensor.matmul(
                                    ps[:, j * 512:(j + 1) * 512],
                                    w_sb[:, hc * QC + ci * 128:
                                         hc * QC + ci * 128 + 128],
                                    xT_sb[:, hc * S + j * 512:
                                          hc * S + j * 512 + 512],
                                    start=(hc == 0), stop=(hc == HC - 1))
                                qk_mms[(wname, ci, hc, j)] = mm
                        hA, hB = 2 * ci, 2 * ci + 1
                        nc.vector.tensor_scalar_add(
                            dst[0:64, hA * S:(hA + 1) * S], ps[0:64, :],
                            b_sb[0:64, ci:ci + 1])
                        nc.vector.tensor_scalar_add(
                            dst[64:128, hB * S:(hB + 1) * S], ps[64:128, :],
                            b_sb[64:128, ci:ci + 1])
                        nc.sync.dma_start(dst[64:128, hA * S:(hA + 1) * S],
                                          dst[0:64, hA * S:(hA + 1) * S])
                        nc.scalar.dma_start(dst[0:64, hB * S:(hB + 1) * S],
                                            dst[64:128, hB * S:(hB + 1) * S])

                # pace the later xT j-blocks and wv behind early Q matmuls
                for j in range(2, TB):
                    for hc in range(HC):
                        tile.add_dep_helper(
                            xt_dmas[(j, hc)].ins,
                            qk_mms[("q", 0, hc, j - 2)].ins,
                            reason="pace xT input load")
                for hc in range(HC):
                    tile.add_dep_helper(wv_dmas[hc].ins,
                                        qk_mms[("q", 0, hc, 1)].ins,
                                        reason="pace wv load")
                for ci in range(2):
                    d = nc.scalar.dma_start(
                        wo_sb[:, ci * HID:(ci + 1) * HID],
                        wo[ci * 128:(ci + 1) * 128, :])
                    tile.add_dep_helper(d.ins, qk_mms[("k", ci, 4, 2)].ins,
                                        reason="pace wo load")

                # V projection in natural [tok, vdim] layout: stationary =
                # xT token-chunk, moving = wv hidden-chunk (all 256 cols).
                # Each [128, 256] psum accumulates over the 8 hidden chunks;
                # a strided DVE copy splits the 4 heads into v_sb strips.
                for tg in range(2):
                    ps = ps1.tile([128, 8 * QC], F32, tag="ps1")
                    for tc8 in range(8):
                        t = tg * 8 + tc8
                        for hc in range(HC):
                            nc.tensor.matmul(
                                ps[:, tc8 * QC:(tc8 + 1) * QC],
                                xT_sb[:, hc * S + t * 128:hc * S + t * 128 + 128],
                                wv_sb[:, hc * QC:(hc + 1) * QC],
                                start=(hc == 0), stop=(hc == HC - 1))
                    for tc8 in range(8):
                        t = tg * 8 + tc8
                        dst = v_sb[:, t * HPC * 128:(t * HPC + HPC) * 128
                                   ].rearrange("p (h e) -> p h e", h=HPC)[:, :, 0:HD]
                        srcv = ps[:, tc8 * QC:(tc8 + 1) * QC
                                  ].rearrange("p (h e) -> p h e", h=HPC)
                        nc.vector.tensor_copy(dst, srcv)

            # ---- phase 2: attention per head ------------------------------
            with (
                tc.tile_pool(name="probs", bufs=5) as probsp,
                tc.tile_pool(name="craw", bufs=2) as crawp,
                tc.tile_pool(name="div", bufs=2) as divp,
                tc.tile_pool(name="scps", bufs=2, space="PSUM") as scps,
                tc.tile_pool(name="ctps", bufs=1, space="PSUM") as ctps,
            ):
                NCP = TC // 2
                # ci0 pair first (earliest ready); last head must be even
                # (the fast final-division path writes ctxf rows 0:64)
                heads = (0, 1, 3, 2)
                stages = [(h, cp) for h in heads for cp in range(NCP)]
                probs_tiles = {}
                ctx_tiles = {}

                def emit_scores(h, cp):
                    hS = h * S
                    c0, c1 = 2 * cp, 2 * cp + 1
                    probs_c = probsp.tile([128, 2 * S], BF16, tag="probs",
                                          name=f"probs_h{h}_cp{cp}")
                    probs_tiles[(h, cp)] = probs_c
                    for j in range(TB):
                        sp = scps.tile([128, 1024], F32, tag="sc")
                        nc.tensor.matmul(
                            sp[:, 0:512],
                            kt2[0:64, hS + c0 * 128:hS + c0 * 128 + 128],
                            qt2[0:64, hS + j * 512:hS + j * 512 + 512],
                            start=True, stop=True)
                        nc.tensor.matmul(
                            sp[:, 512:1024],
                            kt2[64:128, hS + c1 * 128:hS + c1 * 128 + 128],
                            qt2[64:128, hS + j * 512:hS + j * 512 + 512],
                            start=True, stop=True)
                        nc.scalar.activation(
                            probs_c[:, j * 1024:(j + 1) * 1024], sp[:, :], EXP)

                def emit_ctx(h, cp):
                    c0, c1 = 2 * cp, 2 * cp + 1
                    if cp == 0:
                        ctx_tiles[h] = ctps.tile([128, S], F32, tag="ctx",
                                                 name=f"ctx_ps_h{h}")
                    ctx_ps = ctx_tiles[h]
                    probs_c = probs_tiles.pop((h, cp))
                    for j in range(TB):
                        for ck, coff in ((c0, 0), (c1, 512)):
                            vbase = (ck * HPC + h) * 128
                            nc.tensor.matmul(
                                ctx_ps[:, j * 512:(j + 1) * 512],
                                v_sb[:, vbase:vbase + 128],
                                probs_c[:, j * 1024 + coff:
                                        j * 1024 + coff + 512],
                                start=(cp == 0 and ck == c0),
                                stop=(cp == NCP - 1 and ck == c1))

                def emit_division(h):
                    ci = h // 2
                    ctx_ps = ctx_tiles.pop(h)
                    last = (h == heads[-1])
                    if last:
                        craw = crawp.tile([128, S], F32, tag="craw")
                        drow = divp.tile([1, S], F32, tag="drow")
                        nc.scalar.copy(drow[0:1, :], ctx_ps[64:65, :])
                        # j-split so the copies pipeline with the
                        # reciprocal round-trip and the mults below
                        for j in range(TB):
                            jj = slice(j * 512, (j + 1) * 512)
                            nc.vector.tensor_copy(craw[0:64, jj],
                                                  ctx_ps[0:64, jj])
                        denr = divp.tile([128, 16], F32, tag="denr")
                        nc.gpsimd.dma_start(denr[:, :], drow[0:1, :])
                        recr = divp.tile([128, 16], F32, tag="recr")
                        nc.vector.reciprocal(recr[:], denr[:])
                        rrow = divp.tile([1, S], F32, tag="rrow")
                        nc.gpsimd.dma_start(rrow[:, :], recr[:, :])
                        Dt = divp.tile([128, S], F32, tag="Dt")
                        for j in range(TB):
                            jj = slice(j * 512, (j + 1) * 512)
                            nc.gpsimd.partition_broadcast(Dt[:, jj],
                                                          rrow[0:1, jj])
                            nc.vector.tensor_tensor(
                                out=ctxf_sb[0:64, ci * S + j * 512:
                                            ci * S + (j + 1) * 512],
                                in0=craw[0:64, jj], in1=Dt[0:64, jj], op=MULT)
                    else:
                        craw = crawp.tile([128, S], F32, tag="craw")
                        nc.vector.tensor_copy(craw[0:65, :], ctx_ps[0:65, :])
                        denr = divp.tile([128, 16], F32, tag="denr")
                        nc.sync.dma_start(denr[:, :], craw[64:65, :])
                        recr = divp.tile([128, 16], F32, tag="recr")
                        nc.vector.reciprocal(recr[:], denr[:])
                        rrow = divp.tile([1, S], F32, tag="rrow")
                        nc.sync.dma_start(rrow[:, :], recr[:, :])
                        Dt = divp.tile([128, S], F32, tag="Dt")
                        nc.gpsimd.partition_broadcast(Dt[:, :], rrow[0:1, :])
                        if h % 2 == 0:
                            nc.vector.tensor_tensor(
                                out=ctxf_sb[0:64, ci * S:(ci + 1) * S],
                                in0=craw[0:64, :], in1=Dt[0:64, :], op=MULT)
                        else:
                            ctxd = crawp.tile([64, S], BF16, tag="ctxd")
                            nc.vector.tensor_tensor(
                                out=ctxd[0:64, :],
                                in0=craw[0:64, :], in1=Dt[0:64, :], op=MULT)
                            nc.sync.dma_start(
                                ctxf_sb[64:128, ci * S:(ci + 1) * S],
                                ctxd[0:64, :])

                # software pipeline: scores/exp lead ctx by LEAD cp-steps so
                # the PE stream keeps flowing across head boundaries
                LEAD = 2
                for i in range(len(stages) + LEAD):
                    if i < len(stages):
                        emit_scores(*stages[i])
                    if i >= LEAD:
                        h, cp = stages[i - LEAD]
                        emit_ctx(h, cp)
                        if cp == NCP - 1:
                            emit_division(h)

            # ---- phase 3: out projection (partial sums) -------------------
            with (
                tc.tile_pool(name="ops", bufs=8, space="PSUM") as ops,
                tc.tile_pool(name="ostg", bufs=3) as ostg,
            ):
                for t in range(TC):
                    op0 = ops.tile([128, 512], F32, tag="op")
                    op1 = ops.tile([128, 512], F32, tag="op")
                    for ci in range(2):
                        for oc, op in ((0, op0), (1, op1)):
                            nc.tensor.matmul(
                                op[:, :],
                                ctxf_sb[:, ci * S + t * 128:ci * S + t * 128 + 128],
                                wo_sb[:, ci * HID + oc * 512:
                                      ci * HID + oc * 512 + 512],
                                start=(ci == 0), stop=(ci == 1))
                    ot = ostg.tile([128, 1024], BF16, tag="ot")
                    nc.scalar.copy(ot[:, 0:512], op0[:, :])
                    nc.vector.tensor_copy(ot[:, 512:1024], op1[:, :])
                    eng = nc.sync if t % 2 == 0 else nc.scalar
                    eng.dma_start(out[t * 128:(t + 1) * 128, :], ot[:, :])

    nc.compile()
    return nc


_NC = None


def _get_nc():
    global _NC
    if _NC is None:
        _NC = build_nc()
    return _NC


def make_in_maps(x, Wq, bq, Wk, bk, Wv, bv, Wo, bo):
    in_maps = []
    for core in range(NCORES):
        b, g = core // 4, core % 4
        sl = slice(g * QC, (g + 1) * QC)
        in_maps.append({
            "xT": np.ascontiguousarray(x[b].T).astype(np.float16),
            "wq": (np.ascontiguousarray(Wq[:, sl]) * 0.125).astype(np.float16),
            "wk": np.ascontiguousarray(Wk[:, sl]).astype(np.float16),
            "wv": np.ascontiguousarray(Wv[:, sl]).astype(np.float16),
            "wo": np.ascontiguousarray(Wo[sl, :]).astype(ml_dtypes.bfloat16),
            "bq": (np.asarray(bq[sl]) * 0.125).astype(np.float32),
            "bk": np.asarray(bk[sl]).astype(np.float32),
        })
    return in_maps


def combine_outputs(core_outs, Wv_bias_term):
    full = np.empty((B, S, HID), np.float32)
    for b in range(B):
        acc = core_outs[4 * b].astype(np.float32).copy()
        for g in range(1, 4):
            acc += core_outs[4 * b + g]
        full[b] = acc + Wv_bias_term
    return full


def kernel(**inputs):
    x = np.asarray(inputs["x"], np.float32)
    Wq = np.asarray(inputs["Wq"], np.float32)
    bq = np.asarray(inputs["bq"], np.float32)
    Wk = np.asarray(inputs["Wk"], np.float32)
    bk = np.asarray(inputs["bk"], np.float32)
    Wv = np.asarray(inputs["Wv"], np.float32)
    bv = np.asarray(inputs["bv"], np.float32)
    Wo = np.asarray(inputs["Wo"], np.float32)
    bo = np.asarray(inputs["bo"], np.float32)

    nc = _get_nc()
    in_maps = make_in_maps(x, Wq, bq, Wk, bk, Wv, bv, Wo, bo)
    res = run_bass_kernel_spmd(nc, in_maps, core_ids=list(range(NCORES)))
    core_outs = [res.results[c]["out"] for c in range(NCORES)]
    bias_term = (bv @ Wo + bo).astype(np.float32)
    return combine_outputs(core_outs, bias_term)


# revision 22
# speedup vs baseline: 1.0135x; 1.0135x over previous
"""Multi-head attention (B=2, S=2048, H=1024, 16 heads) on 8 TRN2 NeuronCores.

Sharding (tensor-parallel heads x data-parallel batch, per the hint):
  core c -> batch b = c // 4, head group g = c % 4 (4 heads each).
Each core computes, for its batch and its 4 heads:
  Q^T, K^T (transposed layout, qcol on partitions, fp16, duplicated across
  both partition halves), V in natural [tok, vdim] layout (projected
  directly with xT token-chunks as the stationary operand -- no PE
  transpose pass), scores^T = K^T.T @ Q^T per head with two tok_k chunks
  row-tiled concurrently in the two array halves, probs = exp(scores)
  (no max subtraction -- scores ~ N(0,1), bounded), ctx^T via a
  128-column ones-augmented V stationary (cols 65..127 zero) so the
  softmax denominator lands in row 64 of the same accumulation, division
  by the denominator (reciprocal + gpsimd partition broadcast), and the
  partial out-projection ctx^T.T @ Wo_rows.  The 4 partial outputs per
  batch are summed on the host during unsharding (Megatron-style TP
  partial sums).

Phase plan (PE-dense from ~3us to keep the HAM clock warm):
  exp-table preload dummy, then j-major xT DMA so Q-proj starts on the
  first 1 MB; Q/K/V projections back to back; attention pipeline
  (scores+exp LEADs ctx by 2 chunk-pair steps); per-head division;
  pipelined out-projection with the output DMA spread over 3 queues.

Biases: bq/bk applied on-device, bv/bo folded into a host-side additive
constant (bv @ Wo + bo), which is exact.
"""

import ml_dtypes
import numpy as np

import concourse.bacc as bacc
import concourse.mybir as mybir
import concourse.tile as tile
from concourse.bass_utils import run_bass_kernel_spmd

NCORES = 8
B, S, HID = 2, 2048, 1024
NH, HD = 16, 64
HPC = 4            # heads per core
QC = HPC * HD      # 256 local projection cols per core
HC = HID // 128    # 8 hidden chunks
TC = S // 128      # 16 token chunks
TB = S // 512      # 4 token blocks

F32 = mybir.dt.float32
BF16 = mybir.dt.bfloat16
FP16 = mybir.dt.float16
EXP = mybir.ActivationFunctionType.Exp
MULT = mybir.AluOpType.mult


def build_nc():
    nc = bacc.Bacc("TRN2", target_bir_lowering=False, debug=False,
                   num_devices=NCORES)
    xT = nc.declare_dram_parameter("xT", [HID, S], FP16, isOutput=False)
    wq = nc.declare_dram_parameter("wq", [HID, QC], FP16, isOutput=False)
    wk = nc.declare_dram_parameter("wk", [HID, QC], FP16, isOutput=False)
    wv = nc.declare_dram_parameter("wv", [HID, QC], FP16, isOutput=False)
    wo = nc.declare_dram_parameter("wo", [QC, HID], BF16, isOutput=False)
    bq = nc.declare_dram_parameter("bq", [QC], F32, isOutput=False)
    bk = nc.declare_dram_parameter("bk", [QC], F32, isOutput=False)
    out = nc.declare_dram_parameter("out", [S, HID], BF16, isOutput=True)

    with tile.TileContext(nc) as tc:
        with (
            tc.tile_pool(name="const", bufs=1) as constp,
            tc.tile_pool(name="qkv", bufs=1) as qkvp,
        ):
            wo_sb = constp.tile([128, 2 * HID], BF16)
            bq_sb = constp.tile([128, 2], F32)
            bk_sb = constp.tile([128, 2], F32)
            dum_f = constp.tile([1, 16], F32)
            dum_o = constp.tile([1, 16], BF16)
            # preload the exp table set (~2.7us) under the DMA lead-in
            nc.vector.memset(dum_f[:], 0.0)
            nc.scalar.activation(dum_o[:], dum_f[:], EXP)
            # Q^T/K^T per head, duplicated across both partition halves:
            # head h occupies free range [h*S, (h+1)*S) with the same [64, S]
            # data in partitions 0-63 and 64-127, so the scores matmuls can
            # run two tok_k chunks concurrently as row-tiles.
            qt2 = qkvp.tile([128, HPC * S], FP16)
            kt2 = qkvp.tile([128, HPC * S], FP16)
            # Natural V (bf16): per (tok_chunk, head) a 128-wide strip:
            # cols 0..63 = V dims, col 64 = ones (softmax denominator row),
            # cols 65..127 = zeros.  Full-width stationary keeps FWL on.
            v_sb = qkvp.tile([128, TC * HPC * 128], BF16)
            ctxf_sb = qkvp.tile([128, 2 * S], BF16)

            nc.vector.memset(v_sb[:], 0.0)
            for t in range(TC):
                for h in range(HPC):
                    off = (t * HPC + h) * 128 + HD
                    nc.vector.memset(v_sb[:, off:off + 1], 1.0)

            # ---- phase 1: projections -------------------------------------
            with (
                tc.tile_pool(name="xw", bufs=1) as xwp,
                tc.tile_pool(name="ps1", bufs=2, space="PSUM") as ps1,
            ):
                xT_sb = xwp.tile([128, HC * S], FP16)
                wq_sb = xwp.tile([128, HC * QC], FP16)
                wk_sb = xwp.tile([128, HC * QC], FP16)
                wv_sb = xwp.tile([128, HC * QC], FP16)

                # weights for Q first (small), then xT j-major so the
                # j-block-0 slices of all 8 hidden chunks (1 MB) land first
                # and the first Q matmuls can start ~3us in.
                qk_dmas = []
                for hc in range(HC):
                    r = slice(hc * 128, (hc + 1) * 128)
                    qk_dmas.append(nc.sync.dma_start(
                        wq_sb[:, hc * QC:(hc + 1) * QC], wq[r, :]))
                    qk_dmas.append(nc.scalar.dma_start(
                        wk_sb[:, hc * QC:(hc + 1) * QC], wk[r, :]))
                xt_dmas = {}
                for j in range(TB):
                    for hc in range(HC):
                        r = slice(hc * 128, (hc + 1) * 128)
                        eng = nc.sync if hc % 2 == 0 else nc.scalar
                        xt_dmas[(j, hc)] = eng.dma_start(
                            xT_sb[:, hc * S + j * 512:hc * S + (j + 1) * 512],
                            xT[r, j * 512:(j + 1) * 512])
                for ci in range(2):
                    nc.sync.dma_start(bq_sb[:, ci:ci + 1],
                                      bq[ci * 128:(ci + 1) * 128])
                    nc.sync.dma_start(bk_sb[:, ci:ci + 1],
                                      bk[ci * 128:(ci + 1) * 128])
                wv_dmas = []
                for hc in range(HC):
                    r = slice(hc * 128, (hc + 1) * 128)
                    wv_dmas.append(nc.scalar.dma_start(
                        wv_sb[:, hc * QC:(hc + 1) * QC], wv[r, :]))

                # Q then K projections (transposed layout, per ci group),
                # written into the duplicated per-head qt2/kt2 layout.
                qk_mms = {}
                for w_sb, b_sb, dst, wname in ((wq_sb, bq_sb, qt2, "q"),
                                               (wk_sb, bk_sb, kt2, "k")):
                    for ci in range(2):
                        ps = ps1.tile([128, S], F32, tag="ps1")
                        # j-outer: each j-block's 8-chunk accumulation lives
                        # in its own psum bank, and the paced xT j-block
                        # DMAs get a full j-pass (~4.5us) of headroom.
                        for j in range(TB):
                            for hc in range(HC):
                                mm = nc.tensor.matmul(
                                    ps[:, j * 512:(j + 1) * 512],
                                    w_sb[:, hc * QC + ci * 128:
                                         hc * QC + ci * 128 + 128],
                                    xT_sb[:, hc * S + j * 512:
                                          hc * S + j * 512 + 512],
                                    start=(hc == 0), stop=(hc == HC - 1))
                                qk_mms[(wname, ci, hc, j)] = mm
                        hA, hB = 2 * ci, 2 * ci + 1
                        nc.vector.tensor_scalar_add(
                            dst[0:64, hA * S:(hA + 1) * S], ps[0:64, :],
                            b_sb[0:64, ci:ci + 1])
                        nc.vector.tensor_scalar_add(
                            dst[64:128, hB * S:(hB + 1) * S], ps[64:128, :],
                            b_sb[64:128, ci:ci + 1])
                        nc.sync.dma_start(dst[64:128, hA * S:(hA + 1) * S],
                                          dst[0:64, hA * S:(hA + 1) * S])
                        nc.scalar.dma_start(dst[0:64, hB * S:(hB + 1) * S],
                                            dst[64:128, hB * S:(hB + 1) * S])

                # pace the later xT j-blocks and wv behind early Q matmuls
                for j in range(2, TB):
                    for hc in range(HC):
                        tile.add_dep_helper(
                            xt_dmas[(j, hc)].ins,
                            qk_mms[("q", 0, hc, j - 2)].ins,
                            reason="pace xT input load")
                for hc in range(HC):
                    tile.add_dep_helper(wv_dmas[hc].ins,
                                        qk_mms[("q", 0, hc, 1)].ins,
                                        reason="pace wv load")
                for ci in range(2):
                    d = nc.scalar.dma_start(
                        wo_sb[:, ci * HID:(ci + 1) * HID],
                        wo[ci * 128:(ci + 1) * 128, :])
                    tile.add_dep_helper(d.ins, qk_mms[("k", ci, 4, 2)].ins,
                                        reason="pace wo load")

                # V projection in natural [tok, vdim] layout: stationary =
                # xT token-chunk, moving = wv hidden-chunk (all 256 cols).
                # Each [128, 256] psum accumulates over the 8 hidden chunks;
                # a strided DVE copy splits the 4 heads into v_sb strips.
                for tg in range(2):
                    ps = ps1.tile([128, 8 * QC], F32, tag="ps1")
                    for tc8 in range(8):
                        t = tg * 8 + tc8
                        for hc in range(HC):
                            nc.tensor.matmul(
                                ps[:, tc8 * QC:(tc8 + 1) * QC],
                                xT_sb[:, hc * S + t * 128:hc * S + t * 128 + 128],
                                wv_sb[:, hc * QC:(hc + 1) * QC],
                                start=(hc == 0), stop=(hc == HC - 1))
                    for tc8 in range(8):
                        t = tg * 8 + tc8
                        dst = v_sb[:, t * HPC * 128:(t * HPC + HPC) * 128
                                   ].rearrange("p (h e) -> p h e", h=HPC)[:, :, 0:HD]
                        srcv = ps[:, tc8 * QC:(tc8 + 1) * QC
                                  ].rearrange("p (h e) -> p h e", h=HPC)
                        nc.vector.tensor_copy(dst, srcv)

            # ---- phase 2: attention per head ------------------------------
            with (
                tc.tile_pool(name="probs", bufs=5) as probsp,
                tc.tile_pool(name="craw", bufs=2) as crawp,
                tc.tile_pool(name="div", bufs=2) as divp,
                tc.tile_pool(name="scps", bufs=2, space="PSUM") as scps,
                tc.tile_pool(name="ctps", bufs=1, space="PSUM") as ctps,
            ):
                NCP = TC // 2
                # ci0 pair first (earliest ready); last head must be even
                # (the fast final-division path writes ctxf rows 0:64)
                heads = (0, 1, 3, 2)
                stages = [(h, cp) for h in heads for cp in range(NCP)]
                probs_tiles = {}
                ctx_tiles = {}

                def emit_scores(h, cp):
                    hS = h * S
                    c0, c1 = 2 * cp, 2 * cp + 1
                    probs_c = probsp.tile([128, 2 * S], BF16, tag="probs",
                                          name=f"probs_h{h}_cp{cp}")
                    probs_tiles[(h, cp)] = probs_c
                    for j in range(TB):
                        sp = scps.tile([128, 1024], F32, tag="sc")
                        nc.tensor.matmul(
                            sp[:, 0:512],
                            kt2[0:64, hS + c0 * 128:hS + c0 * 128 + 128],
                            qt2[0:64, hS + j * 512:hS + j * 512 + 512],
                            start=True, stop=True)
                        nc.tensor.matmul(
                            sp[:, 512:1024],
                            kt2[64:128, hS + c1 * 128:hS + c1 * 128 + 128],
                            qt2[64:128, hS + j * 512:hS + j * 512 + 512],
                            start=True, stop=True)
                        nc.scalar.activation(
                            probs_c[:, j * 1024:(j + 1) * 1024], sp[:, :], EXP)

                def emit_ctx(h, cp):
                    c0, c1 = 2 * cp, 2 * cp + 1
                    if cp == 0:
                        ctx_tiles[h] = ctps.tile([128, S], F32, tag="ctx",
                                                 name=f"ctx_ps_h{h}")
                    ctx_ps = ctx_tiles[h]
                    probs_c = probs_tiles.pop((h, cp))
                    for j in range(TB):
                        for ck, coff in ((c0, 0), (c1, 512)):
                            vbase = (ck * HPC + h) * 128
                            nc.tensor.matmul(
                                ctx_ps[:, j * 512:(j + 1) * 512],
                                v_sb[:, vbase:vbase + 128],
                                probs_c[:, j * 1024 + coff:
                                        j * 1024 + coff + 512],
                                start=(cp == 0 and ck == c0),
                                stop=(cp == NCP - 1 and ck == c1))

                def emit_division(h):
                    ci = h // 2
                    ctx_ps = ctx_tiles.pop(h)
                    last = (h == heads[-1])
                    if last:
                        craw = crawp.tile([128, S], F32, tag="craw")
                        drow = divp.tile([1, S], F32, tag="drow")
                        nc.scalar.copy(drow[0:1, :], ctx_ps[64:65, :])
                        # j-split so the copies pipeline with the
                        # reciprocal round-trip and the mults below
                        for j in range(TB):
                            jj = slice(j * 512, (j + 1) * 512)
                            nc.vector.tensor_copy(craw[0:64, jj],
                                                  ctx_ps[0:64, jj])
                        denr = divp.tile([128, 16], F32, tag="denr")
                        nc.gpsimd.dma_start(denr[:, :], drow[0:1, :])
                        recr = divp.tile([128, 16], F32, tag="recr")
                        nc.vector.reciprocal(recr[:], denr[:])
                        rrow = divp.tile([1, S], F32, tag="rrow")
                        nc.gpsimd.dma_start(rrow[:, :], recr[:, :])
                        Dt = divp.tile([128, S], F32, tag="Dt")
                        for j in range(TB):
                            jj = slice(j * 512, (j + 1) * 512)
                            nc.gpsimd.partition_broadcast(Dt[:, jj],
                                                          rrow[0:1, jj])
                            nc.vector.tensor_tensor(
                                out=ctxf_sb[0:64, ci * S + j * 512:
                                            ci * S + (j + 1) * 512],
                                in0=craw[0:64, jj], in1=Dt[0:64, jj], op=MULT)
                    else:
                        craw = crawp.tile([128, S], F32, tag="craw")
                        nc.vector.tensor_copy(craw[0:65, :], ctx_ps[0:65, :])
                        denr = divp.tile([128, 16], F32, tag="denr")
                        nc.sync.dma_start(denr[:, :], craw[64:65, :])
                        recr = divp.tile([128, 16], F32, tag="recr")
                        nc.vector.reciprocal(recr[:], denr[:])
                        rrow = divp.tile([1, S], F32, tag="rrow")
                        nc.sync.dma_start(rrow[:, :], recr[:, :])
                        Dt = divp.tile([128, S], F32, tag="Dt")
                        nc.gpsimd.partition_broadcast(Dt[:, :], rrow[0:1, :])
                        if h % 2 == 0:
                            nc.vector.tensor_tensor(
                                out=ctxf_sb[0:64, ci * S:(ci + 1) * S],
                                in0=craw[0:64, :], in1=Dt[0:64, :], op=MULT)
                        else:
                            ctxd = crawp.tile([64, S], BF16, tag="ctxd")
                            nc.vector.tensor_tensor(
                                out=ctxd[0:64, :],
                                in0=craw[0:64, :], in1=Dt[0:64, :], op=MULT)
                            nc.sync.dma_start(
                                ctxf_sb[64:128, ci * S:(ci + 1) * S],
                                ctxd[0:64, :])

                # software pipeline: scores/exp lead ctx by LEAD cp-steps so
                # the PE stream keeps flowing across head boundaries
                LEAD = 3
                for i in range(len(stages) + LEAD):
                    if i < len(stages):
                        emit_scores(*stages[i])
                    if i >= LEAD:
                        h, cp = stages[i - LEAD]
                        emit_ctx(h, cp)
                        if cp == NCP - 1:
                            emit_division(h)

            # ---- phase 3: out projection (partial sums) -------------------
            with (
                tc.tile_pool(name="ops", bufs=8, space="PSUM") as ops,
                tc.tile_pool(name="ostg", bufs=3) as ostg,
            ):
                for t in range(TC):
                    op0 = ops.tile([128, 512], F32, tag="op")
                    op1 = ops.tile([128, 512], F32, tag="op")
                    for ci in range(2):
                        for oc, op in ((0, op0), (1, op1)):
                            nc.tensor.matmul(
                                op[:, :],
                                ctxf_sb[:, ci * S + t * 128:ci * S + t * 128 + 128],
                                wo_sb[:, ci * HID + oc * 512:
                                      ci * HID + oc * 512 + 512],
                                start=(ci == 0), stop=(ci == 1))
                    ot = ostg.tile([128, 1024], BF16, tag="ot")
                    nc.scalar.copy(ot[:, 0:512], op0[:, :])
                    nc.vector.tensor_copy(ot[:, 512:1024], op1[:, :])
                    eng = nc.sync if t % 2 == 0 else nc.scalar
                    eng.dma_start(out[t * 128:(t + 1) * 128, :], ot[:, :])

    nc.compile()
    return nc


_NC = None


def _get_nc():
    global _NC
    if _NC is None:
        _NC = build_nc()
    return _NC


def make_in_maps(x, Wq, bq, Wk, bk, Wv, bv, Wo, bo):
    in_maps = []
    for core in range(NCORES):
        b, g = core // 4, core % 4
        sl = slice(g * QC, (g + 1) * QC)
        in_maps.append({
            "xT": np.ascontiguousarray(x[b].T).astype(np.float16),
            "wq": (np.ascontiguousarray(Wq[:, sl]) * 0.125).astype(np.float16),
            "wk": np.ascontiguousarray(Wk[:, sl]).astype(np.float16),
            "wv": np.ascontiguousarray(Wv[:, sl]).astype(np.float16),
            "wo": np.ascontiguousarray(Wo[sl, :]).astype(ml_dtypes.bfloat16),
            "bq": (np.asarray(bq[sl]) * 0.125).astype(np.float32),
            "bk": np.asarray(bk[sl]).astype(np.float32),
        })
    return in_maps


def combine_outputs(core_outs, Wv_bias_term):
    full = np.empty((B, S, HID), np.float32)
    for b in range(B):
        acc = core_outs[4 * b].astype(np.float32).copy()
        for g in range(1, 4):
            acc += core_outs[4 * b + g]
        full[b] = acc + Wv_bias_term
    return full


def kernel(**inputs):
    x = np.asarray(inputs["x"], np.float32)
    Wq = np.asarray(inputs["Wq"], np.float32)
    bq = np.asarray(inputs["bq"], np.float32)
    Wk = np.asarray(inputs["Wk"], np.float32)
    bk = np.asarray(inputs["bk"], np.float32)
    Wv = np.asarray(inputs["Wv"], np.float32)
    bv = np.asarray(inputs["bv"], np.float32)
    Wo = np.asarray(inputs["Wo"], np.float32)
    bo = np.asarray(inputs["bo"], np.float32)

    nc = _get_nc()
    in_maps = make_in_maps(x, Wq, bq, Wk, bk, Wv, bv, Wo, bo)
    res = run_bass_kernel_spmd(nc, in_maps, core_ids=list(range(NCORES)))
    core_outs = [res.results[c]["out"] for c in range(NCORES)]
    bias_term = (bv @ Wo + bo).astype(np.float32)
    return combine_outputs(core_outs, bias_term)
